# revision 24
# baseline (speedup 1.0000x reference)
"""Trainium2 Bass kernel for tied-row MSA attention (nn_Attention_52329881535135).

Strategy (8 NeuronCores, one chip):
  - Shard the MSA row dim r (leading b*r=256) across the 8 cores: 32 rows each.
  - Host pre-transposes x to xT[r, D, N] in bf16, so the device does plain
    (non-transposing) DMA loads and every matmul runs in bf16.
  - Phase 1 in two blocks of 8 row-pairs: q/k projections for the block into a
    pair-stacked per-head layout, then per-head tied logits dotsT[h,j,i]
    accumulated over the block's pairs in PSUM (block 0 parks partials in
    SBUF bf16).  Block 1 adds the partials and flushes each head as it
    completes -> two 4-head bf16 AllReduces pipeline behind the remaining
    dots compute.
  - Phase 2: v projections (the PE filler for the AR tail) staged in SBUF and
    spilled to DRAM (1MB per pair; SBUF cannot hold 16 pairs), per-head
    softmax as each AR lands (exp on ACT, column sums via ones-matmul,
    reciprocal folded into the exp tiles in place on gpsimd), then a single
    attention sweep per pair (all 8 heads + output projection, v reloaded
    from DRAM with depth-2 prefetch), written once.
  - PSUM evacuations alternate vector/scalar (gpsimd has no PSUM port);
    gpsimd carries the AR-gated dl loads so they never head-block an evac
    queue.

  Mask bookkeeping (has_rows / num_rows / mask_any) is computed on the host at
  call time and folded into the weights / an additive column bias, so the
  device graph only does dense matmuls.
"""

import sys

sys.path.insert(0, "/opt/trn_rl_repo")

import numpy as np

B, R, N, D, H, DH = 1, 256, 512, 256, 8, 64
INNER = H * DH
NCORES = 8
R_LOC = R // NCORES  # 32 rows per core
P = 128
NPT = N // P  # 4 position tiles
NJT = N // P  # 4 j tiles
NDT = D // P  # 2 d tiles
PAIRS = R_LOC // 2  # 16
NBLK = 2
BPAIRS = PAIRS // NBLK  # 8 pairs per phase-1 block

_graph_cache = {}


def _build(
    separate_xq: bool,
    has_bias: bool = True,
    n_cores: int = NCORES,
    do_finalize: bool = True,
):
    from contextlib import ExitStack

    from concourse import bacc, mybir, tile

    f32 = mybir.dt.float32
    bf16 = mybir.dt.bfloat16
    AF = mybir.ActivationFunctionType
    ALU = mybir.AluOpType

    nc = bacc.Bacc(
        "TRN2", target_bir_lowering=False, debug=False, num_devices=n_cores
    )

    xt_ext = nc.declare_dram_parameter("xt", [R_LOC, D, N], bf16, isOutput=False)
    if separate_xq:
        xqt_ext = nc.declare_dram_parameter("xqt", [R_LOC, D, N], bf16, isOutput=False)
    wq_ext = nc.declare_dram_parameter("Wq", [D, INNER], bf16, isOutput=False)
    wk_ext = nc.declare_dram_parameter("Wk", [D, INNER], bf16, isOutput=False)
    wv_ext = nc.declare_dram_parameter("Wv", [D, INNER], bf16, isOutput=False)
    wo_ext = nc.declare_dram_parameter("Wo", [INNER, D], bf16, isOutput=False)
    bo_ext = nc.declare_dram_parameter("bo", [D], f32, isOutput=False)
    jb_ext = nc.declare_dram_parameter("jbias", [NJT, P], f32, isOutput=False)
    out_ext = nc.declare_dram_parameter("out", [R_LOC, N, D], f32, isOutput=True)

    # per-half-head logits AllReduce, carried in bf16
    out_space = "Shared" if n_cores > 4 else "Local"
    cc_in = nc.dram_tensor("cc_in", [H, P, NJT, N], bf16)
    cc_out = nc.dram_tensor("cc_out", [H, P, NJT, N], bf16, addr_space=out_space)
    # v2 spill: v projections round-trip DRAM (1MB per pair)
    v2_dram = nc.dram_tensor("v2_dram", [PAIRS, P, NJT, H, 2, DH], bf16)

    with tile.TileContext(nc) as tc, ExitStack() as top:
        consts = top.enter_context(tc.tile_pool(name="consts", bufs=1))
        xt_pool = top.enter_context(tc.tile_pool(name="xt", bufs=6))

        # --- evacuation engine rotation (PSUM reads: DVE + ACT only) ---
        evac_engines = [nc.vector, nc.scalar]
        evac_idx = [0]

        def rot_copy(out, in_):
            e = evac_engines[evac_idx[0] % len(evac_engines)]
            evac_idx[0] += 1
            if e is nc.scalar:
                return e.copy(out, in_)
            return e.tensor_copy(out=out, in_=in_)

        # --- constants / weights (already bf16 in DRAM) ---
        wq_sb = consts.tile([P, NDT, INNER], bf16, name="wq_sb")
        wk_sb = consts.tile([P, NDT, INNER], bf16, name="wk_sb")
        for dt in range(NDT):
            nc.gpsimd.dma_start(
                wq_sb[:, dt, :],
                wq_ext.rearrange("(o p) f -> p o f", p=P)[:, dt, :],
            )
            nc.scalar.dma_start(
                wk_sb[:, dt, :],
                wk_ext.rearrange("(o p) f -> p o f", p=P)[:, dt, :],
            )
        wv_sb = consts.tile([P, NDT, INNER], bf16, name="wv_sb")
        nc.gpsimd.dma_start(wv_sb[:], wv_ext.rearrange("(o p) f -> p o f", p=P))
        wo_sb = consts.tile([P, NPT, D], bf16, name="wo_sb")
        nc.scalar.dma_start(wo_sb[:], wo_ext.rearrange("(o p) e -> p o e", p=P))

        ones_col = consts.tile([P, 1], bf16, name="ones_col")
        nc.any.memset(ones_col, 1.0)
        ones_row = consts.tile([1, P], bf16, name="ones_row")
        nc.any.memset(ones_row, 1.0)
        jb_sb = consts.tile([P, NJT], f32, name="jb_sb")
        nc.sync.dma_start(jb_sb[:], jb_ext.rearrange("t p -> p t"))
        if has_bias:
            ones_row_f = consts.tile([1, P], f32, name="ones_row_f")
            nc.any.memset(ones_row_f, 1.0)
            bo_sb = consts.tile([1, D], f32, name="bo_sb")
            nc.sync.dma_start(bo_sb[:], bo_ext[None, :])
            bo_bcast = consts.tile([P, D], f32, name="bo_bcast")
            with tc.tile_pool(name="initpsum", bufs=1, space="PSUM") as initp:
                bp0 = initp.tile([P, D], f32, name="bp0")
                nc.tensor.matmul(
                    bp0[:], ones_row_f[:], bo_sb[:], start=True, stop=True
                )
                nc.any.tensor_copy(out=bo_bcast[:], in_=bp0[:])

        def load_xT(src_ext, r, tag):
            xT = xt_pool.tile([P, NDT, N], bf16, tag=f"xT_{tag}")
            nc.sync.dma_start(xT[:], src_ext[r].rearrange("(o p) n -> p o n", p=P))
            return xT

        v2_pool = None
        vpsum = None

        def emit_v(pair):
            """v projection for one row pair -> pair-stacked [j, (r0.hd|r1.hd)]
            staged in SBUF, spilled to DRAM."""
            v2 = v2_pool.tile([P, NJT, H, 2, DH], bf16, tag="v2", name=f"v2_{pair}")
            for parity in range(2):
                r = 2 * pair + parity
                xT = load_xT(xt_ext, r, "p2")
                for pt in range(NPT):
                    vp = vpsum.tile([P, INNER], f32, tag="vp")
                    for dt in range(NDT):
                        nc.tensor.matmul(
                            vp[:],
                            xT[:, dt, pt * P : (pt + 1) * P],
                            wv_sb[:, dt, :],
                            start=(dt == 0),
                            stop=(dt == NDT - 1),
                        )
                    rot_copy(
                        v2[:, pt, :, parity, :],
                        vp.rearrange("p (h d) -> p h d", h=H),
                    )
            nc.sync.dma_start(v2_dram[pair], v2[:])

        # =====================  Phase 1: q/k + tied dots  =====================
        with ExitStack() as ph1:
            qk_pool = ph1.enter_context(tc.tile_pool(name="qk", bufs=1))
            dsb_pool = ph1.enter_context(tc.tile_pool(name="dsb", bufs=1))
            proj_psum = ph1.enter_context(
                tc.tile_pool(name="proj_psum", bufs=4, space="PSUM")
            )
            dots_psum = ph1.enter_context(
                tc.tile_pool(name="dots_psum", bufs=2, space="PSUM")
            )
            ccsb_pool = ph1.enter_context(tc.tile_pool(name="ccsb", bufs=8))

            # bf16 partial dots from block 0 (psum can't persist across blocks)
            dots_sb = dsb_pool.tile([P, H, NJT, N], bf16, name="dots_sb")

            for blk in range(NBLK):
                q2 = qk_pool.tile([P, H, BPAIRS, N], bf16, tag="q2")
                k2 = qk_pool.tile([P, H, BPAIRS, N], bf16, tag="k2")
                for pq in range(BPAIRS):
                    for parity in range(2):
                        r = (blk * BPAIRS + pq) * 2 + parity
                        xT = load_xT(xt_ext, r, "p1")
                        if separate_xq:
                            xTq = load_xT(xqt_ext, r, "p1q")
                        else:
                            xTq = xT
                        off = 64 * parity
                        for wsb, xtt, dest in (
                            (wq_sb, xTq, q2),
                            (wk_sb, xT, k2),
                        ):
                            for t in range(NPT):
                                pp = proj_psum.tile([P, N], f32, tag="pp")
                                for dt in range(NDT):
                                    nc.tensor.matmul(
                                        pp[:],
                                        wsb[:, dt, t * P : (t + 1) * P],
                                        xtt[:, dt, :],
                                        start=(dt == 0),
                                        stop=(dt == NDT - 1),
                                    )
                                rot_copy(
                                    dest[off : off + 64, 2 * t, pq, :],
                                    pp[0:64, :],
                                )
                                rot_copy(
                                    dest[off : off + 64, 2 * t + 1, pq, :],
                                    pp[64:128, :],
                                )
                # tied logits for this block: dotsT[h, j, i] += pair sums
                for h in range(H):
                    for jt in range(NJT):
                        dp = dots_psum.tile([P, N], f32, tag="dp")
                        for pq in range(BPAIRS):
                            nc.tensor.matmul(
                                dp[:],
                                k2[:, h, pq, jt * P : (jt + 1) * P],
                                q2[:, h, pq, :],
                                start=(pq == 0),
                                stop=(pq == BPAIRS - 1),
                            )
                        if blk == 0:
                            rot_copy(dots_sb[:, h, jt, :], dp[:])
                        else:
                            cc_t = ccsb_pool.tile([P, N], bf16, tag="ccsb")
                            nc.vector.tensor_tensor(
                                cc_t[:], dp[:], dots_sb[:, h, jt, :], ALU.add
                            )
                            nc.gpsimd.dma_start(cc_in[h][:, jt, :], cc_t[:])
                    if blk == NBLK - 1 and h in (3, H - 1):
                        hs = slice(0, 4) if h == 3 else slice(4, H)
                        nc.gpsimd.collective_compute(
                            "AllReduce",
                            ALU.add,
                            replica_groups=[list(range(n_cores))],
                            ins=[cc_in[hs]],
                            outs=[cc_out[hs]],
                        )

        # ============  Phase 2: v (AR filler), softmax, attn, out  ===========
        with ExitStack() as ph2:
            v2_pool = ph2.enter_context(tc.tile_pool(name="v2p", bufs=3))
            v2r_pool = ph2.enter_context(tc.tile_pool(name="v2r", bufs=4))
            exp_pool = ph2.enter_context(tc.tile_pool(name="expp", bufs=1))
            rs_pool = ph2.enter_context(tc.tile_pool(name="rsp", bufs=1))
            dl_pool = ph2.enter_context(tc.tile_pool(name="dlp", bufs=2))
            sm_pool = ph2.enter_context(tc.tile_pool(name="smp", bufs=2))
            out2_pool = ph2.enter_context(tc.tile_pool(name="o2p", bufs=3))
            yout_pool = ph2.enter_context(tc.tile_pool(name="yout", bufs=4))
            attpsum = None
            ypsum = None
            smpsum = None

            # raw exp; 1/colsum is applied in place on gpsimd
            exp_sb = exp_pool.tile([P, H, NJT, N], bf16, name="exp_sb")
            rs_bcast = rs_pool.tile([P, H, N], f32, name="rs_bcast")

            def sm_exp(h):
                """dl + exp for one head (dl rides gpsimd: no other queue
                head-blocks on the AR gate)."""
                dl = dl_pool.tile([P, NJT, N], bf16, tag="dl")
                nc.gpsimd.dma_start(dl[:], cc_out[h][:])
                for jt in range(NJT):
                    nc.scalar.activation(
                        exp_sb[:, h, jt, :],
                        dl[:, jt, :],
                        AF.Exp,
                        bias=jb_sb[:, jt : jt + 1],
                        scale=1.0,
                    )

            def sm_sum(h):
                """column sums + reciprocal; exp scaled in place on gpsimd."""
                sp = smpsum.tile([1, N], f32, tag="sm")
                for jt in range(NJT):
                    nc.tensor.matmul(
                        sp[:],
                        ones_col[:],
                        exp_sb[:, h, jt, :],
                        start=(jt == 0),
                        stop=(jt == NJT - 1),
                    )
                s_bf = sm_pool.tile([1, N], bf16, tag="s_bf")
                nc.vector.tensor_copy(out=s_bf[:], in_=sp[:])
                bps = smpsum.tile([P, N], f32, tag="sm")
                nc.tensor.matmul(bps[:], ones_row[:], s_bf[:], start=True, stop=True)
                nc.vector.reciprocal(rs_bcast[:, h, :], bps[:])
                for jt in range(NJT):
                    nc.gpsimd.tensor_tensor(
                        exp_sb[:, h, jt, :],
                        exp_sb[:, h, jt, :],
                        rs_bcast[:, h, :],
                        ALU.mult,
                    )

            def reload_v(pair):
                v2r = v2r_pool.tile(
                    [P, NJT, H, 2, DH], bf16, tag="v2r", name=f"v2r_{pair}"
                )
                nc.sync.dma_start(v2r[:], v2_dram[pair])
                return v2r

            def attn_full(pair, v2r):
                """attn @ v for all heads of one pair, then the output
                projection, written once."""
                out2 = [
                    out2_pool.tile(
                        [P, NPT, N], bf16, tag=f"o2_{par}", name=f"o2_{par}_{pair}"
                    )
                    for par in range(2)
                ]
                for h in range(H):
                    ap_ps = attpsum.tile([P, N], f32, tag="att")
                    for jt in range(NJT):
                        nc.tensor.matmul(
                            ap_ps[:],
                            v2r[:, jt, h, :, :],
                            exp_sb[:, h, jt, :],
                            start=(jt == 0),
                            stop=(jt == NJT - 1),
                        )
                    t, sub = h // 2, h % 2
                    for parity in range(2):
                        o = 64 * parity
                        rot_copy(
                            out2[parity][sub * 64 : sub * 64 + 64, t, :],
                            ap_ps[o : o + 64, :],
                        )
                for parity in range(2):
                    r = 2 * pair + parity
                    dst = out_ext[r].rearrange("(po pi) e -> pi po e", pi=P)
                    for it in range(NPT):
                        yp = ypsum.tile([P, D], f32, tag="yp")
                        for t in range(NPT):
                            nc.tensor.matmul(
                                yp[:],
                                out2[parity][:, t, it * P : (it + 1) * P],
                                wo_sb[:, t, :],
                                start=(t == 0),
                                stop=(t == NPT - 1),
                            )
                        yo = yout_pool.tile([P, D], f32, tag="yo")
                        if has_bias:
                            nc.vector.tensor_tensor(
                                yo[:], yp[:], bo_bcast[:], ALU.add
                            )
                        else:
                            rot_copy(yo[:], yp[:])
                        eng = nc.gpsimd if (it + parity) % 2 == 0 else nc.sync
                        eng.dma_start(dst[:, it, :], yo[:])

            # ---- phase-2 emission schedule ----
            # v-sweep + softmax use their own PSUM scope; the attention
            # sweep then gets 4+4 banks for deeper matmul/evac pipelining
            with ExitStack() as ph2a:
                vpsum = ph2a.enter_context(
                    tc.tile_pool(name="vpsum", bufs=3, space="PSUM")
                )
                smpsum = ph2a.enter_context(
                    tc.tile_pool(name="smpsum", bufs=2, space="PSUM")
                )
                for i in range(PAIRS):
                    emit_v(i)
                    if 4 <= i <= 7:
                        sm_exp(i - 4)
                        sm_sum(i - 4)
                    if 10 <= i <= 13:
                        sm_exp(i - 6)
                for h in range(4, H):
                    sm_sum(h)
            with ExitStack() as ph2b:
                attpsum = ph2b.enter_context(
                    tc.tile_pool(name="attpsum", bufs=4, space="PSUM")
                )
                ypsum = ph2b.enter_context(
                    tc.tile_pool(name="ypsum", bufs=4, space="PSUM")
                )
                v2rs = {}
                for p in range(2):
                    v2rs[p] = reload_v(p)
                for p in range(PAIRS):
                    if p + 2 < PAIRS:
                        v2rs[p + 2] = reload_v(p + 2)
                    attn_full(p, v2rs.pop(p))

    if do_finalize:
        nc.finalize()
    return nc


def _get_graph(separate_xq: bool, has_bias: bool):
    key = (separate_xq, has_bias)
    if key not in _graph_cache:
        _graph_cache[key] = _build(separate_xq, has_bias)
    return _graph_cache[key]


def _prepare(x, mask, Wq, Wk, Wv, Wo, bo, tie_attn_dim):
    """Host-side prep: mask bookkeeping, weight folding, bf16 transpose,
    sharded in_maps."""
    import ml_dtypes

    bf16 = ml_dtypes.bfloat16

    x = np.asarray(x, dtype=np.float32)
    mask = np.asarray(mask).astype(bool)
    Wq = np.asarray(Wq, dtype=np.float32)
    Wk = np.asarray(Wk, dtype=np.float32)
    Wv = np.asarray(Wv, dtype=np.float32)
    Wo = np.asarray(Wo, dtype=np.float32)
    bo = np.ascontiguousarray(np.asarray(bo, dtype=np.float32))
    r = int(tie_attn_dim)
    assert x.shape == (B * R, N, D) and r == R, (x.shape, r)

    m = mask.reshape(B, R, N)
    has_rows = m.any(axis=-1)[0]  # [R]
    num_rows = max(int(has_rows.sum()), 1)
    col_valid = m.any(axis=1)[0]  # [N]

    scale = (DH ** -0.5) * (num_rows ** -0.5)
    Wq_eff = np.ascontiguousarray((Wq * np.float32(scale)).astype(bf16))
    Wk_b = np.ascontiguousarray(Wk.astype(bf16))
    Wv_b = np.ascontiguousarray(Wv.astype(bf16))
    Wo_b = np.ascontiguousarray(Wo.astype(bf16))

    jbias = np.where(col_valid, 0.0, -1e30).astype(np.float32)
    jbias = np.ascontiguousarray(jbias.reshape(NJT, P))

    has_bias = bool(np.any(bo != 0.0))
    separate_xq = not bool(has_rows.all())

    # [b*r, N, D] -> [b*r, D, N] bf16 (device reads plain, pre-transposed)
    xt = np.ascontiguousarray(x.transpose(0, 2, 1).astype(bf16))
    if separate_xq:
        xq = x * has_rows[:, None, None].astype(np.float32)
        xqt = np.ascontiguousarray(xq.transpose(0, 2, 1).astype(bf16))

    in_maps = []
    for c in range(NCORES):
        im = {
            "xt": xt[c * R_LOC : (c + 1) * R_LOC],
            "Wq": Wq_eff,
            "Wk": Wk_b,
            "Wv": Wv_b,
            "Wo": Wo_b,
            "bo": bo,
            "jbias": jbias,
        }
        if separate_xq:
            im["xqt"] = xqt[c * R_LOC : (c + 1) * R_LOC]
        in_maps.append(im)
    return separate_xq, has_bias, in_maps


def kernel(x, mask, Wq, Wk, Wv, Wo, bo, tie_attn_dim):
    from concourse.bass_utils import run_bass_kernel_spmd

    separate_xq, has_bias, in_maps = _prepare(
        x, mask, Wq, Wk, Wv, Wo, bo, tie_attn_dim
    )
    nc = _get_graph(separate_xq, has_bias)
    res = run_bass_kernel_spmd(nc, in_maps, list(range(NCORES)))
    out = np.concatenate([res.results[c]["out"] for c in range(NCORES)], axis=0)
    return out.astype(np.float32)


def _install_ntff_hook():
    """The agent image's antenv lacks axon_hooks; recreate it so trace=True
    can drive NTFF profiling through libaxon_pjrt.so (see trn_boot.py)."""
    try:
        from antenv import axon_hooks  # noqa: F401

        return
    except ImportError:
        pass
    import types

    import antenv

    mod = types.ModuleType("antenv.axon_hooks")
    holder = {}
    mod.set_axon_ntff_profile_hook = lambda h: holder.__setitem__("h", h)
    mod.get_axon_ntff_profile_hook = lambda: holder.get("h")
    sys.modules["antenv.axon_hooks"] = mod
    antenv.axon_hooks = mod
    if "/root/.axon_site" not in sys.path:
        sys.path.insert(0, "/root/.axon_site")
    from trn_agent_boot.trn_boot import _ntff_profile_via_ctypes

    mod.set_axon_ntff_profile_hook(
        _ntff_profile_via_ctypes("/opt/axon/libaxon_pjrt.so")
    )


def bench(inputs):
    """Run with neuron-profile tracing; returns (BassKernelResults, output)."""
    from concourse.bass_utils import run_bass_kernel_spmd

    _install_ntff_hook()
    separate_xq, has_bias, in_maps = _prepare(**inputs)
    nc = _get_graph(separate_xq, has_bias)
    res = run_bass_kernel_spmd(nc, in_maps, list(range(NCORES)), trace=True)
    out = np.concatenate([res.results[c]["out"] for c in range(NCORES)], axis=0)
    return res, out.astype(np.float32)


# revision 25
# speedup vs baseline: 1.0182x; 1.0182x over previous
"""Trainium2 Bass kernel for tied-row MSA attention (nn_Attention_52329881535135).

Strategy (8 NeuronCores, one chip):
  - Shard the MSA row dim r (leading b*r=256) across the 8 cores: 32 rows each.
  - Host pre-transposes x to xT[r, D, N] in bf16, so the device does plain
    (non-transposing) DMA loads and every matmul runs in bf16.
  - Phase 1 in two blocks of 8 row-pairs: q/k projections for the block into a
    pair-stacked per-head layout, then per-head tied logits dotsT[h,j,i]
    accumulated over the block's pairs in PSUM (block 0 parks partials in
    SBUF bf16).  Block 1 adds the partials and flushes each head as it
    completes -> two 4-head bf16 AllReduces pipeline behind the remaining
    dots compute.
  - Phase 2: v projections (the PE filler for the AR tail) staged in SBUF and
    spilled to DRAM (1MB per pair; SBUF cannot hold 16 pairs), per-head
    softmax as each AR lands (exp on ACT, column sums via ones-matmul,
    reciprocal folded into the exp tiles in place on gpsimd), then a single
    attention sweep per pair (all 8 heads + output projection, v reloaded
    from DRAM with depth-2 prefetch), written once.
  - PSUM evacuations alternate vector/scalar (gpsimd has no PSUM port);
    gpsimd carries the AR-gated dl loads so they never head-block an evac
    queue.

  Mask bookkeeping (has_rows / num_rows / mask_any) is computed on the host at
  call time and folded into the weights / an additive column bias, so the
  device graph only does dense matmuls.
"""

import sys

sys.path.insert(0, "/opt/trn_rl_repo")

import numpy as np

B, R, N, D, H, DH = 1, 256, 512, 256, 8, 64
INNER = H * DH
NCORES = 8
R_LOC = R // NCORES  # 32 rows per core
P = 128
NPT = N // P  # 4 position tiles
NJT = N // P  # 4 j tiles
NDT = D // P  # 2 d tiles
PAIRS = R_LOC // 2  # 16
NBLK = 2
BPAIRS = PAIRS // NBLK  # 8 pairs per phase-1 block

_graph_cache = {}


def _build(
    separate_xq: bool,
    has_bias: bool = True,
    n_cores: int = NCORES,
    do_finalize: bool = True,
):
    from contextlib import ExitStack

    from concourse import bacc, mybir, tile

    f32 = mybir.dt.float32
    bf16 = mybir.dt.bfloat16
    AF = mybir.ActivationFunctionType
    ALU = mybir.AluOpType

    nc = bacc.Bacc(
        "TRN2", target_bir_lowering=False, debug=False, num_devices=n_cores
    )

    xt_ext = nc.declare_dram_parameter("xt", [R_LOC, D, N], bf16, isOutput=False)
    if separate_xq:
        xqt_ext = nc.declare_dram_parameter("xqt", [R_LOC, D, N], bf16, isOutput=False)
    wq_ext = nc.declare_dram_parameter("Wq", [D, INNER], bf16, isOutput=False)
    wk_ext = nc.declare_dram_parameter("Wk", [D, INNER], bf16, isOutput=False)
    wv_ext = nc.declare_dram_parameter("Wv", [D, INNER], bf16, isOutput=False)
    wo_ext = nc.declare_dram_parameter("Wo", [INNER, D], bf16, isOutput=False)
    bo_ext = nc.declare_dram_parameter("bo", [D], f32, isOutput=False)
    jb_ext = nc.declare_dram_parameter("jbias", [NJT, P], f32, isOutput=False)
    out_ext = nc.declare_dram_parameter("out", [R_LOC, N, D], f32, isOutput=True)

    # per-half-head logits AllReduce, carried in bf16
    out_space = "Shared" if n_cores > 4 else "Local"
    cc_in = nc.dram_tensor("cc_in", [H, P, NJT, N], bf16)
    cc_out = nc.dram_tensor("cc_out", [H, P, NJT, N], bf16, addr_space=out_space)
    # v2 spill: v projections round-trip DRAM (1MB per pair)
    v2_dram = nc.dram_tensor("v2_dram", [PAIRS, P, NJT, H, 2, DH], bf16)

    with tile.TileContext(nc) as tc, ExitStack() as top:
        consts = top.enter_context(tc.tile_pool(name="consts", bufs=1))
        xt_pool = top.enter_context(tc.tile_pool(name="xt", bufs=6))

        # --- evacuation engine rotation (PSUM reads: DVE + ACT only) ---
        evac_engines = [nc.vector, nc.scalar]
        evac_idx = [0]

        def rot_copy(out, in_):
            e = evac_engines[evac_idx[0] % len(evac_engines)]
            evac_idx[0] += 1
            if e is nc.scalar:
                return e.copy(out, in_)
            return e.tensor_copy(out=out, in_=in_)

        # --- constants / weights (already bf16 in DRAM) ---
        wq_sb = consts.tile([P, NDT, INNER], bf16, name="wq_sb")
        wk_sb = consts.tile([P, NDT, INNER], bf16, name="wk_sb")
        for dt in range(NDT):
            nc.gpsimd.dma_start(
                wq_sb[:, dt, :],
                wq_ext.rearrange("(o p) f -> p o f", p=P)[:, dt, :],
            )
            nc.scalar.dma_start(
                wk_sb[:, dt, :],
                wk_ext.rearrange("(o p) f -> p o f", p=P)[:, dt, :],
            )
        wv_sb = consts.tile([P, NDT, INNER], bf16, name="wv_sb")
        nc.gpsimd.dma_start(wv_sb[:], wv_ext.rearrange("(o p) f -> p o f", p=P))
        wo_sb = consts.tile([P, NPT, D], bf16, name="wo_sb")
        nc.scalar.dma_start(wo_sb[:], wo_ext.rearrange("(o p) e -> p o e", p=P))

        ones_col = consts.tile([P, 1], bf16, name="ones_col")
        nc.any.memset(ones_col, 1.0)
        ones_row = consts.tile([1, P], bf16, name="ones_row")
        nc.any.memset(ones_row, 1.0)
        jb_sb = consts.tile([P, NJT], f32, name="jb_sb")
        nc.sync.dma_start(jb_sb[:], jb_ext.rearrange("t p -> p t"))
        if has_bias:
            ones_row_f = consts.tile([1, P], f32, name="ones_row_f")
            nc.any.memset(ones_row_f, 1.0)
            bo_sb = consts.tile([1, D], f32, name="bo_sb")
            nc.sync.dma_start(bo_sb[:], bo_ext[None, :])
            bo_bcast = consts.tile([P, D], f32, name="bo_bcast")
            with tc.tile_pool(name="initpsum", bufs=1, space="PSUM") as initp:
                bp0 = initp.tile([P, D], f32, name="bp0")
                nc.tensor.matmul(
                    bp0[:], ones_row_f[:], bo_sb[:], start=True, stop=True
                )
                nc.any.tensor_copy(out=bo_bcast[:], in_=bp0[:])

        def load_xT(src_ext, r, tag):
            xT = xt_pool.tile([P, NDT, N], bf16, tag=f"xT_{tag}")
            nc.sync.dma_start(xT[:], src_ext[r].rearrange("(o p) n -> p o n", p=P))
            return xT

        v2_pool = None
        vpsum = None

        def emit_v(pair):
            """v projection for one row pair -> pair-stacked [j, (r0.hd|r1.hd)]
            staged in SBUF, spilled to DRAM."""
            v2 = v2_pool.tile([P, NJT, H, 2, DH], bf16, tag="v2", name=f"v2_{pair}")
            for parity in range(2):
                r = 2 * pair + parity
                xT = load_xT(xt_ext, r, "p2")
                for pt in range(NPT):
                    vp = vpsum.tile([P, INNER], f32, tag="vp")
                    for dt in range(NDT):
                        nc.tensor.matmul(
                            vp[:],
                            xT[:, dt, pt * P : (pt + 1) * P],
                            wv_sb[:, dt, :],
                            start=(dt == 0),
                            stop=(dt == NDT - 1),
                        )
                    rot_copy(
                        v2[:, pt, :, parity, :],
                        vp.rearrange("p (h d) -> p h d", h=H),
                    )
            nc.sync.dma_start(v2_dram[pair], v2[:])

        # =====================  Phase 1: q/k + tied dots  =====================
        with ExitStack() as ph1:
            qk_pool = ph1.enter_context(tc.tile_pool(name="qk", bufs=1))
            dsb_pool = ph1.enter_context(tc.tile_pool(name="dsb", bufs=1))
            proj_psum = ph1.enter_context(
                tc.tile_pool(name="proj_psum", bufs=4, space="PSUM")
            )
            dots_psum = ph1.enter_context(
                tc.tile_pool(name="dots_psum", bufs=2, space="PSUM")
            )
            ccsb_pool = ph1.enter_context(tc.tile_pool(name="ccsb", bufs=8))

            # bf16 partial dots from block 0 (psum can't persist across blocks)
            dots_sb = dsb_pool.tile([P, H, NJT, N], bf16, name="dots_sb")

            for blk in range(NBLK):
                q2 = qk_pool.tile([P, H, BPAIRS, N], bf16, tag="q2")
                k2 = qk_pool.tile([P, H, BPAIRS, N], bf16, tag="k2")
                for pq in range(BPAIRS):
                    for parity in range(2):
                        r = (blk * BPAIRS + pq) * 2 + parity
                        xT = load_xT(xt_ext, r, "p1")
                        if separate_xq:
                            xTq = load_xT(xqt_ext, r, "p1q")
                        else:
                            xTq = xT
                        off = 64 * parity
                        for wsb, xtt, dest in (
                            (wq_sb, xTq, q2),
                            (wk_sb, xT, k2),
                        ):
                            for t in range(NPT):
                                pp = proj_psum.tile([P, N], f32, tag="pp")
                                for dt in range(NDT):
                                    nc.tensor.matmul(
                                        pp[:],
                                        wsb[:, dt, t * P : (t + 1) * P],
                                        xtt[:, dt, :],
                                        start=(dt == 0),
                                        stop=(dt == NDT - 1),
                                    )
                                rot_copy(
                                    dest[off : off + 64, 2 * t, pq, :],
                                    pp[0:64, :],
                                )
                                rot_copy(
                                    dest[off : off + 64, 2 * t + 1, pq, :],
                                    pp[64:128, :],
                                )
                # tied logits for this block: dotsT[h, j, i] += pair sums
                for h in range(H):
                    for jt in range(NJT):
                        dp = dots_psum.tile([P, N], f32, tag="dp")
                        for pq in range(BPAIRS):
                            nc.tensor.matmul(
                                dp[:],
                                k2[:, h, pq, jt * P : (jt + 1) * P],
                                q2[:, h, pq, :],
                                start=(pq == 0),
                                stop=(pq == BPAIRS - 1),
                            )
                        if blk == 0:
                            rot_copy(dots_sb[:, h, jt, :], dp[:])
                        else:
                            cc_t = ccsb_pool.tile([P, N], bf16, tag="ccsb")
                            nc.vector.tensor_tensor(
                                cc_t[:], dp[:], dots_sb[:, h, jt, :], ALU.add
                            )
                            nc.gpsimd.dma_start(cc_in[h][:, jt, :], cc_t[:])
                    if blk == NBLK - 1 and h in (3, H - 1):
                        hs = slice(0, 4) if h == 3 else slice(4, H)
                        nc.gpsimd.collective_compute(
                            "AllReduce",
                            ALU.add,
                            replica_groups=[list(range(n_cores))],
                            ins=[cc_in[hs]],
                            outs=[cc_out[hs]],
                        )

        # ============  Phase 2: v (AR filler), softmax, attn, out  ===========
        with ExitStack() as ph2:
            v2_pool = ph2.enter_context(tc.tile_pool(name="v2p", bufs=3))
            v2r_pool = ph2.enter_context(tc.tile_pool(name="v2r", bufs=4))
            exp_pool = ph2.enter_context(tc.tile_pool(name="expp", bufs=1))
            rs_pool = ph2.enter_context(tc.tile_pool(name="rsp", bufs=1))
            dl_pool = ph2.enter_context(tc.tile_pool(name="dlp", bufs=2))
            sm_pool = ph2.enter_context(tc.tile_pool(name="smp", bufs=2))
            out2_pool = ph2.enter_context(tc.tile_pool(name="o2p", bufs=3))
            yout_pool = ph2.enter_context(tc.tile_pool(name="yout", bufs=4))
            attpsum = None
            ypsum = None
            smpsum = None

            # raw exp; 1/colsum is applied in place on gpsimd
            exp_sb = exp_pool.tile([P, H, NJT, N], bf16, name="exp_sb")
            rs_bcast = rs_pool.tile([P, H, N], f32, name="rs_bcast")

            def sm_exp(h):
                """dl + exp for one head (dl rides gpsimd: no other queue
                head-blocks on the AR gate)."""
                dl = dl_pool.tile([P, NJT, N], bf16, tag="dl")
                nc.gpsimd.dma_start(dl[:], cc_out[h][:])
                for jt in range(NJT):
                    nc.scalar.activation(
                        exp_sb[:, h, jt, :],
                        dl[:, jt, :],
                        AF.Exp,
                        bias=jb_sb[:, jt : jt + 1],
                        scale=1.0,
                    )

            def sm_sum(h):
                """column sums + reciprocal; exp scaled in place on gpsimd."""
                sp = smpsum.tile([1, N], f32, tag="sm")
                for jt in range(NJT):
                    nc.tensor.matmul(
                        sp[:],
                        ones_col[:],
                        exp_sb[:, h, jt, :],
                        start=(jt == 0),
                        stop=(jt == NJT - 1),
                    )
                s_bf = sm_pool.tile([1, N], bf16, tag="s_bf")
                nc.vector.tensor_copy(out=s_bf[:], in_=sp[:])
                bps = smpsum.tile([P, N], f32, tag="sm")
                nc.tensor.matmul(bps[:], ones_row[:], s_bf[:], start=True, stop=True)
                nc.vector.reciprocal(rs_bcast[:, h, :], bps[:])
                for jt in range(NJT):
                    nc.gpsimd.tensor_tensor(
                        exp_sb[:, h, jt, :],
                        exp_sb[:, h, jt, :],
                        rs_bcast[:, h, :],
                        ALU.mult,
                    )

            def reload_v(pair):
                v2r = v2r_pool.tile(
                    [P, NJT, H, 2, DH], bf16, tag="v2r", name=f"v2r_{pair}"
                )
                nc.sync.dma_start(v2r[:], v2_dram[pair])
                return v2r

            def attn_full(pair, v2r):
                """attn @ v for all heads of one pair, then the output
                projection, written once."""
                out2 = [
                    out2_pool.tile(
                        [P, NPT, N], bf16, tag=f"o2_{par}", name=f"o2_{par}_{pair}"
                    )
                    for par in range(2)
                ]
                for h in range(H):
                    ap_ps = attpsum.tile([P, N], f32, tag="att")
                    for jt in range(NJT):
                        nc.tensor.matmul(
                            ap_ps[:],
                            v2r[:, jt, h, :, :],
                            exp_sb[:, h, jt, :],
                            start=(jt == 0),
                            stop=(jt == NJT - 1),
                        )
                    t, sub = h // 2, h % 2
                    for parity in range(2):
                        o = 64 * parity
                        rot_copy(
                            out2[parity][sub * 64 : sub * 64 + 64, t, :],
                            ap_ps[o : o + 64, :],
                        )
                for parity in range(2):
                    r = 2 * pair + parity
                    dst = out_ext[r].rearrange("(po pi) e -> pi po e", pi=P)
                    for it in range(NPT):
                        yp = ypsum.tile([P, D], f32, tag="yp")
                        for t in range(NPT):
                            nc.tensor.matmul(
                                yp[:],
                                out2[parity][:, t, it * P : (it + 1) * P],
                                wo_sb[:, t, :],
                                start=(t == 0),
                                stop=(t == NPT - 1),
                            )
                        yo = yout_pool.tile([P, D], f32, tag="yo")
                        if has_bias:
                            nc.vector.tensor_tensor(
                                yo[:], yp[:], bo_bcast[:], ALU.add
                            )
                        else:
                            rot_copy(yo[:], yp[:])
                        nc.gpsimd.dma_start(dst[:, it, :], yo[:])

            # ---- phase-2 emission schedule ----
            # v-sweep + softmax use their own PSUM scope; the attention
            # sweep then gets 4+4 banks for deeper matmul/evac pipelining
            with ExitStack() as ph2a:
                vpsum = ph2a.enter_context(
                    tc.tile_pool(name="vpsum", bufs=3, space="PSUM")
                )
                smpsum = ph2a.enter_context(
                    tc.tile_pool(name="smpsum", bufs=2, space="PSUM")
                )
                for i in range(PAIRS):
                    emit_v(i)
                    if 4 <= i <= 7:
                        sm_exp(i - 4)
                        sm_sum(i - 4)
                    if 10 <= i <= 13:
                        sm_exp(i - 6)
                for h in range(4, H):
                    sm_sum(h)
            with ExitStack() as ph2b:
                attpsum = ph2b.enter_context(
                    tc.tile_pool(name="attpsum", bufs=4, space="PSUM")
                )
                ypsum = ph2b.enter_context(
                    tc.tile_pool(name="ypsum", bufs=4, space="PSUM")
                )
                v2rs = {}
                for p in range(2):
                    v2rs[p] = reload_v(p)
                for p in range(PAIRS):
                    if p + 2 < PAIRS:
                        v2rs[p + 2] = reload_v(p + 2)
                    attn_full(p, v2rs.pop(p))

    if do_finalize:
        nc.finalize()
    return nc


def _get_graph(separate_xq: bool, has_bias: bool):
    key = (separate_xq, has_bias)
    if key not in _graph_cache:
        _graph_cache[key] = _build(separate_xq, has_bias)
    return _graph_cache[key]


def _prepare(x, mask, Wq, Wk, Wv, Wo, bo, tie_attn_dim):
    """Host-side prep: mask bookkeeping, weight folding, bf16 transpose,
    sharded in_maps."""
    import ml_dtypes

    bf16 = ml_dtypes.bfloat16

    x = np.asarray(x, dtype=np.float32)
    mask = np.asarray(mask).astype(bool)
    Wq = np.asarray(Wq, dtype=np.float32)
    Wk = np.asarray(Wk, dtype=np.float32)
    Wv = np.asarray(Wv, dtype=np.float32)
    Wo = np.asarray(Wo, dtype=np.float32)
    bo = np.ascontiguousarray(np.asarray(bo, dtype=np.float32))
    r = int(tie_attn_dim)
    assert x.shape == (B * R, N, D) and r == R, (x.shape, r)

    m = mask.reshape(B, R, N)
    has_rows = m.any(axis=-1)[0]  # [R]
    num_rows = max(int(has_rows.sum()), 1)
    col_valid = m.any(axis=1)[0]  # [N]

    scale = (DH ** -0.5) * (num_rows ** -0.5)
    Wq_eff = np.ascontiguousarray((Wq * np.float32(scale)).astype(bf16))
    Wk_b = np.ascontiguousarray(Wk.astype(bf16))
    Wv_b = np.ascontiguousarray(Wv.astype(bf16))
    Wo_b = np.ascontiguousarray(Wo.astype(bf16))

    jbias = np.where(col_valid, 0.0, -1e30).astype(np.float32)
    jbias = np.ascontiguousarray(jbias.reshape(NJT, P))

    has_bias = bool(np.any(bo != 0.0))
    separate_xq = not bool(has_rows.all())

    # [b*r, N, D] -> [b*r, D, N] bf16 (device reads plain, pre-transposed)
    xt = np.ascontiguousarray(x.transpose(0, 2, 1).astype(bf16))
    if separate_xq:
        xq = x * has_rows[:, None, None].astype(np.float32)
        xqt = np.ascontiguousarray(xq.transpose(0, 2, 1).astype(bf16))

    in_maps = []
    for c in range(NCORES):
        im = {
            "xt": xt[c * R_LOC : (c + 1) * R_LOC],
            "Wq": Wq_eff,
            "Wk": Wk_b,
            "Wv": Wv_b,
            "Wo": Wo_b,
            "bo": bo,
            "jbias": jbias,
        }
        if separate_xq:
            im["xqt"] = xqt[c * R_LOC : (c + 1) * R_LOC]
        in_maps.append(im)
    return separate_xq, has_bias, in_maps


def kernel(x, mask, Wq, Wk, Wv, Wo, bo, tie_attn_dim):
    from concourse.bass_utils import run_bass_kernel_spmd

    separate_xq, has_bias, in_maps = _prepare(
        x, mask, Wq, Wk, Wv, Wo, bo, tie_attn_dim
    )
    nc = _get_graph(separate_xq, has_bias)
    res = run_bass_kernel_spmd(nc, in_maps, list(range(NCORES)))
    out = np.concatenate([res.results[c]["out"] for c in range(NCORES)], axis=0)
    return out.astype(np.float32)


def _install_ntff_hook():
    """The agent image's antenv lacks axon_hooks; recreate it so trace=True
    can drive NTFF profiling through libaxon_pjrt.so (see trn_boot.py)."""
    try:
        from antenv import axon_hooks  # noqa: F401

        return
    except ImportError:
        pass
    import types

    import antenv

    mod = types.ModuleType("antenv.axon_hooks")
    holder = {}
    mod.set_axon_ntff_profile_hook = lambda h: holder.__setitem__("h", h)
    mod.get_axon_ntff_profile_hook = lambda: holder.get("h")
    sys.modules["antenv.axon_hooks"] = mod
    antenv.axon_hooks = mod
    if "/root/.axon_site" not in sys.path:
        sys.path.insert(0, "/root/.axon_site")
    from trn_agent_boot.trn_boot import _ntff_profile_via_ctypes

    mod.set_axon_ntff_profile_hook(
        _ntff_profile_via_ctypes("/opt/axon/libaxon_pjrt.so")
    )


def bench(inputs):
    """Run with neuron-profile tracing; returns (BassKernelResults, output)."""
    from concourse.bass_utils import run_bass_kernel_spmd

    _install_ntff_hook()
    separate_xq, has_bias, in_maps = _prepare(**inputs)
    nc = _get_graph(separate_xq, has_bias)
    res = run_bass_kernel_spmd(nc, in_maps, list(range(NCORES)), trace=True)
    out = np.concatenate([res.results[c]["out"] for c in range(NCORES)], axis=0)
    return res, out.astype(np.float32)
